# revision 1
# baseline (speedup 1.0000x reference)
"""TRN2 Bass kernel for OneLayerCNN: conv2d(4x4, stride 2, pad 2) + bias + ReLU.

Input  A_prev (64, 256, 256, 3) f32, W (4,4,3,16), b (1,1,1,16)
Output (64, 129*129*16) f32.

Data-parallel over 8 NeuronCores (8 images each). Per core, per h-block of
15 output rows:
  - row-pair tiles: partition (re, img) holds input rows (2re, 2re+1) as one
    6KB contiguous DMA descriptor; parities are column halves of the tile.
  - PE transposes (is_transpose matmul vs identity) turn [instance, offset]
    windows into [offset, instance] SBUF tiles (contraction on partitions).
  - conv = per w-block 4 accumulating float32r matmuls: stationary lhsT =
    transposed-activation window [K<=121, M=120 instances], moving rhs =
    host-precomputed banded weights [121, 304 = 19 w' x 16 cout]. The bias
    rides a ones-row at K=kw on the fh=0 matmul; border w-blocks use
    K-truncated windows with host-shifted weight variants (no zero padding).
  - fused ReLU on PSUM eviction (DVE/ACT), one contiguous-run output DMA
    per h-block (8KB+ descriptors).
A post-pass splits multi-sem-wait instructions (this walrus accepts one
sync wait per instruction). A short PE warmup during the initial DMA wait
opens the HAM clock gate before the real matmuls.
"""
import numpy as np
from contextlib import ExitStack

import concourse.bass as bass
import concourse.tile as tile
from concourse import mybir
from concourse.bass_utils import run_bass_kernel_spmd
import bass_rust

# ---------------- problem constants (hardcoded) ----------------
N_CORES = 8
IMG = 8              # images per core
H = 256
WID = 256
CIN = 3
F = 4
COUT = 16
HO = 129
WO = 129
RW = WID * CIN       # 768 floats per row
NH_FULL = 15         # h' rows per full block
NB = 9               # 8 full blocks + 1 ragged (9 h')
WBLK = 19            # w' per w-block (B=0..5), B=6 computes 16, keeps 15
NWB = 7
KW = 120             # banded K window (6*18+12)
KB = 121             # K incl bias row
NMM = WBLK * COUT    # 304
OUTROW = WO * COUT   # 2064

DT_MM = mybir.dt.float32r   # matmul dtype knob (float32r | float32)
DT_F32 = mybir.dt.float32


def _split_multi_waits(nc):
    """walrus here accepts at most ONE sync wait per instruction; hoist
    extras onto NoOps inserted just before, same engine queue."""
    ctr = 0
    for f in nc.m.functions:
        for bb in f.blocks:
            insts = bb.instructions  # live list
            out = []
            changed = False
            for inst in insts:
                si = inst.sync_info
                if si is None:
                    out.append(inst)
                    continue
                waits = list(si.on_wait)
                if len(waits) > 1:
                    changed = True
                    for w in waits[:-1]:
                        ctr += 1
                        nop = mybir.InstNoOp(name=f"I-wsplit-{ctr}")
                        nop.engine = inst.engine
                        nop.sync_info = bass_rust.SyncInfo(
                            on_wait=[w], on_update=[])
                        out.append(nop)
                    inst.sync_info = bass_rust.SyncInfo(
                        on_wait=[waits[-1]], on_update=list(si.on_update))
                out.append(inst)
            if changed:
                insts[:] = out
    return nc


def _make_wband(W_arr, b_arr):
    """4 banded weight mats [121, 304]: wb[fh][6s+3fw+ci, 16s+co] = W[fh,fw,ci,co];
    wb[0][120, 16s+co] = b[co]. Plus two bias-carrying edge variants for the
    first/last w-blocks whose K windows are truncated at the image border:
    wb0_e0 [115, 304] = [wb0[6:120]; bias], wb0_e6 [91, 256] = [wb0[0:90]; bias]."""
    wbs = []
    for fh in range(F):
        wb = np.zeros((KB, NMM), dtype=np.float32)
        for s in range(WBLK):
            for fw in range(F):
                for ci in range(CIN):
                    wb[6 * s + 3 * fw + ci, 16 * s:16 * s + 16] = \
                        W_arr[fh, fw, ci, :]
        if fh == 0:
            for s in range(WBLK):
                wb[120, 16 * s:16 * s + 16] = b_arr.reshape(-1)
        wbs.append(wb)
    # B=0 edge: all four taps shifted to row 0 (SBUF operands must start at
    # an aligned base partition, so slices [6:120] are precomputed on host)
    e0 = [np.concatenate([wbs[0][6:120], wbs[0][120:121]], axis=0)]  # [115,304]
    for fh in range(1, F):
        e0.append(wbs[fh][6:120].copy())                             # [114,304]
    e6 = np.concatenate([wbs[0][0:90, 0:240],
                         wbs[0][120:121, 0:240]], axis=0)            # [91,240]
    # pack weights into one [128, CONSTW] tensor (one DMA); identity ships
    # separately so the first transposes are not gated on the big transfer
    blocks = e0 + wbs + [e6]
    comb = np.zeros((128, sum(a.shape[1] for a in blocks)), dtype=np.float32)
    col = 0
    for a in blocks:
        comb[0:a.shape[0], col:col + a.shape[1]] = a
        col += a.shape[1]
    return comb, np.eye(128, dtype=np.float32)


def _build_nc(dt_mm=DT_MM):
    nc = bass.Bass()
    a_in = nc.declare_dram_parameter("A", [IMG, H, RW], dt_mm, isOutput=False)
    CONSTW = NMM * 8 + 240
    c_in = nc.declare_dram_parameter("consts", [128, CONSTW], dt_mm,
                                     isOutput=False)
    id_in = nc.declare_dram_parameter("ident", [128, 128], dt_mm,
                                      isOutput=False)
    z_out = nc.declare_dram_parameter("Z", [IMG, HO, OUTROW], DT_F32,
                                      isOutput=True)

    # row pairs: apair[re][img] = rows (2re, 2re+1) concatenated, 6KB each
    apair = a_in.rearrange("i (re two) c -> re i (two c)", two=2)

    # per-w-block geometry: window start col, K width, matmul N, evicted N
    #   B=0 and B=6 have border-truncated K windows (no zero padding needed;
    #   dropped K rows correspond exactly to the conv's zero pads)
    BGEO = []
    for B in range(NWB):
        win = max(0, 114 * B - 6)
        kw = min(RW, 114 * B - 6 + KW) - win     # 114 | 120 | 90
        nmm = NMM if B < 6 else 240              # B=6: 15 w' (no junk cols;
        ncols = NMM if B < 6 else 240            # f32r has no N<256 penalty)
        BGEO.append((win, kw, nmm, ncols))

    with tile.TileContext(nc) as tc, ExitStack() as ctx:
        consts = ctx.enter_context(tc.tile_pool(name="consts", bufs=1))
        rpool = ctx.enter_context(tc.tile_pool(name="rows", bufs=4))
        tpool = ctx.enter_context(tc.tile_pool(name="tsb", bufs=6))
        opool = ctx.enter_context(tc.tile_pool(name="oacc", bufs=2))
        pt_pool = ctx.enter_context(
            tc.tile_pool(name="ptr", bufs=4, space="PSUM"))
        pc_pool = ctx.enter_context(
            tc.tile_pool(name="pconv", bufs=4, space="PSUM"))

        # PE warmup: ~5us of dummy matmuls during the initial DMA wait so the
        # HAM clock gate opens (1.2 -> 2.4 GHz) before the real work starts
        wtile = consts.tile([128, 640], dt_mm, tag="wtile", name="wtile")
        nc.gpsimd.memset(wtile[:].bitcast(DT_F32), 0.0)
        pwarm = pt_pool.tile([128, 512], DT_F32, tag="ptr", name="pwarm")
        for _ in range(6):
            nc.tensor.matmul(pwarm[:], wtile[0:128, 0:128],
                             wtile[0:128, 128:640], start=True, stop=True)

        # identity first (tiny, gates the first transposes), then one DMA
        # for the weight set; both on the scalar HWDGE queue so the row
        # loads (sync queue) are not stuck behind them
        ident = consts.tile([128, 128], dt_mm, tag="ident", name="ident")
        nc.scalar.dma_start(out=ident[:], in_=id_in[:])
        call = consts.tile([128, CONSTW], dt_mm, tag="call", name="call")
        nc.scalar.dma_start(out=call[:], in_=c_in[:])
        off = 0
        wbe0 = []
        for fh in range(F):
            wbe0.append(call[0:(115 if fh == 0 else 114), off:off + NMM])
            off += NMM
        wb_sb = []
        for fh in range(F):
            wb_sb.append(call[0:KB, off:off + NMM])
            off += NMM
        wbe6 = call[0:91, off:off + 240]
        ident_ap = ident[:]

        for b in range(NB):
            h0 = NH_FULL * b
            nh = NH_FULL if b < NB - 1 else HO - NH_FULL * (NB - 1)  # 15 | 9
            nl = nh + 1          # parity rows needed: re = h0-1 .. h0+nh-1
            m = nh * IMG         # matmul M (120 | 72)

            # one tile holds both parities: partition (re,img) = rows
            # (2re, 2re+1) back to back -> one 6KB descriptor per partition
            rp = rpool.tile([128, 2 * RW], dt_mm, tag="rp", name="rp")
            l0, l1 = 0, nl
            if b == 0:
                l0 = 1                      # re = -1 is a zero row pair
                nc.gpsimd.memset(rp[0:8, :].bitcast(DT_F32), 0.0)
            if b == NB - 1:
                l1 = nl - 1                 # re = 128 is a zero row pair
                # 32-aligned base; rows below (nl-1)*8 are re-loaded by
                # the DMA below, which follows in program order (WAW)
                nc.gpsimd.memset(rp[64:128, :].bitcast(DT_F32), 0.0)
            re0 = h0 - 1 + l0
            asrc = apair[re0:re0 + (l1 - l0)]
            if b == 0:
                # stage the B=0 windows (both parities) first so the first
                # transposes can start as early as possible
                nc.sync.dma_start(out=rp[l0 * 8:l1 * 8, 0:128],
                                  in_=asrc[:, :, 0:128])
                nc.sync.dma_start(out=rp[l0 * 8:l1 * 8, 768:896],
                                  in_=asrc[:, :, 768:896])
                nc.sync.dma_start(out=rp[l0 * 8:l1 * 8, 128:768],
                                  in_=asrc[:, :, 128:768])
                nc.sync.dma_start(out=rp[l0 * 8:l1 * 8, 896:2 * RW],
                                  in_=asrc[:, :, 896:2 * RW])
            else:
                nc.sync.dma_start(out=rp[l0 * 8:l1 * 8, :], in_=asrc)

            oacc = opool.tile([128, OUTROW], DT_F32, tag="oacc")
            for B in range(NWB):
                win, kw, nmm, ncols = BGEO[B]
                # rhs weights for the four fh taps (B=0/6 use row slices)
                r0 = win - (114 * B - 6)         # 6 at B=0 else 0
                if B == 0:
                    wrhs0 = wbe0[0][0:kw + 1, 0:nmm]
                    wrhs = [wbe0[fh][0:kw, 0:nmm] for fh in range(1, F)]
                elif B == 6:
                    wrhs0 = wbe6[0:kw + 1, 0:nmm]
                    wrhs = [wb_sb[fh][0:kw, 0:nmm] for fh in range(1, F)]
                else:
                    wrhs0 = wb_sb[0][0:kw + 1, 0:nmm]
                    wrhs = [wb_sb[fh][0:kw, 0:nmm] for fh in range(1, F)]
                ones_base = (kw // 32) * 32      # 32-aligned memset base
                tsb = []
                for p in range(2):
                    ptr = pt_pool.tile([kw, 128], dt_mm, tag="ptr", name="ptr")
                    nc.tensor.transpose(
                        ptr[:], rp[:, RW * p + win:RW * p + win + kw],
                        ident_ap)
                    t = tpool.tile([KB, 128], dt_mm, tag=f"t{p}", name=f"tsb{p}")
                    if p == 0:
                        # ones row at partition kw (bias): memset a 32-aligned
                        # range; the evict below overwrites rows < kw
                        nc.gpsimd.memset(
                            t[ones_base:kw + 1, :].bitcast(DT_F32), 1.0)
                    nc.vector.tensor_copy(t[0:kw, :], ptr[:])
                    tsb.append(t)
                pc = pc_pool.tile([128, NMM], DT_F32, tag="pc")
                nc.tensor.matmul(pc[0:m, 0:nmm], tsb[0][0:kw + 1, 0:m],
                                 wrhs0, start=True, stop=False)
                nc.tensor.matmul(pc[0:m, 0:nmm], tsb[1][0:kw, 0:m],
                                 wrhs[0], start=False, stop=False)
                nc.tensor.matmul(pc[0:m, 0:nmm], tsb[0][0:kw, 8:8 + m],
                                 wrhs[1], start=False, stop=False)
                nc.tensor.matmul(pc[0:m, 0:nmm], tsb[1][0:kw, 8:8 + m],
                                 wrhs[2], start=False, stop=True)
                # ReLU eviction: alternate ACT/DVE to balance engines
                if B % 3 == 2:
                    nc.scalar.activation(
                        oacc[0:m, 304 * B:304 * B + ncols], pc[0:m, 0:ncols],
                        mybir.ActivationFunctionType.Relu)
                else:
                    nc.vector.tensor_scalar_max(
                        oacc[0:m, 304 * B:304 * B + ncols],
                        pc[0:m, 0:ncols], 0.0)

            dst = z_out[:, h0:h0 + nh, :].rearrange("i j c -> j i c")
            nc.scalar.dma_start(out=dst, in_=oacc[0:m, :])

    _split_multi_waits(nc)
    return nc


_NC_CACHE = {}


def _get_nc(dt_mm=DT_MM):
    key = str(dt_mm)
    if key not in _NC_CACHE:
        _NC_CACHE[key] = _build_nc(dt_mm)
    return _NC_CACHE[key]


def kernel(A_prev, W, b, _trace=False, _dt=None):
    A_prev = np.ascontiguousarray(A_prev, dtype=np.float32)
    W = np.asarray(W, dtype=np.float32)
    b = np.asarray(b, dtype=np.float32)
    comb, ident = _make_wband(W, b)

    nc = _get_nc(_dt or DT_MM)
    in_maps = []
    for c in range(N_CORES):
        shard = A_prev[c * IMG:(c + 1) * IMG].reshape(IMG, H, RW)
        in_maps.append({"A": shard, "consts": comb, "ident": ident})

    res = run_bass_kernel_spmd(nc, in_maps, list(range(N_CORES)),
                               trace=_trace)
    out = np.concatenate([res.results[c]["Z"].reshape(IMG, -1)
                          for c in range(N_CORES)], axis=0)
    if _trace:
        return out, res
    return out



# revision 2
# speedup vs baseline: 1.3539x; 1.3539x over previous
"""TRN2 Bass kernel for OneLayerCNN: conv2d(4x4, stride 2, pad 2) + bias + ReLU.

Input  A_prev (64, 256, 256, 3) f32, W (4,4,3,16), b (1,1,1,16)
Output (64, 129*129*16) f32.

Data-parallel over 8 NeuronCores (8 images each). v2 design:

- Host pre-packs the input into matmul-ready fp16 "column strips": row PAIRS
  (2re, 2re+1) are column-interleaved (c = 2*(3x+ci) + rowparity), so one
  K<=121 band window spans TWO filter rows -> only 2 accumulating matmuls
  per output block (vs 4 with single-row banding), and the strips arrive
  transposed ([band-offset, (pair,img)]) so there are ZERO PE transposes
  and zero PSUM->SBUF transpose copies.
- 15 w-blocks of 9 outputs (K=120+bias row, N=9*16=144), 9 h-blocks of 16
  rows (M=128 instances = 16 h' x 8 img). Stationary operand = activation
  window (fp16 -> fast weight load), moving = banded weights.
- Bias rides a host-baked ones-row at partition K of each strip (tap-0
  matmul only). Zero pads are K-truncations of border blocks with
  host-shifted weight variants.
- fp16 everywhere off-chip (half the HBM traffic of f32); PSUM accumulates
  fp32; ReLU fused into the PSUM eviction (DVE/ACT alternating); output is
  written fp16 and upcast to f32 on the host.
A post-pass splits multi-sem-wait instructions (walrus accepts one sync
wait per instruction). A short PE warmup during the initial DMA wait opens
the HAM clock gate before the real matmuls.
"""
import numpy as np
from contextlib import ExitStack

import concourse.bass as bass
import concourse.tile as tile
from concourse import mybir
from concourse.bass_utils import run_bass_kernel_spmd
import bass_rust

# ---------------- problem constants (hardcoded) ----------------
N_CORES = 8
IMG = 8              # images per core
H = 256
WID = 256
CIN = 3
F = 4
COUT = 16
HO = 129
WO = 129
RW = WID * CIN       # 768 floats per row
IC = 2 * RW          # 1536: row-pair interleaved width
S = 9                # w' outputs per w-block
NWB = 15             # w-blocks (14 full + 1 of 3 outputs)
NPAIR = 130          # row pairs incl. re=-1 and re=128 zero pads
NI = NPAIR * IMG     # 1040 instance columns per strip
NHB = 9              # h-blocks: 8 x 16 h' + 1 x 1 h'
OUTROW = WO * COUT   # 2064
QS = [0, 264, 528, 792, 1040]   # instance-quarter boundaries (x8 h'-pairs)

DT = mybir.dt.float16
DT32 = mybir.dt.float32


def _bgeo(B):
    """w-block geometry: (c0 = strip window start in interleaved coords,
    K = band rows, N = matmul cols)."""
    c0 = max(0, 108 * B - 12)
    c1 = min(IC, 108 * B + 108)
    ns = min(S, WO - S * B)
    return c0, c1 - c0, ns * COUT


def _split_multi_waits(nc):
    """walrus accepts at most ONE sync wait per instruction; hoist extras
    onto NoOps inserted just before, same engine queue."""
    ctr = 0
    for f in nc.m.functions:
        for bb in f.blocks:
            insts = bb.instructions  # live list
            out = []
            changed = False
            for inst in insts:
                si = inst.sync_info
                if si is None:
                    out.append(inst)
                    continue
                waits = list(si.on_wait)
                if len(waits) > 1:
                    changed = True
                    for w in waits[:-1]:
                        ctr += 1
                        nop = mybir.InstNoOp(name=f"I-wsplit-{ctr}")
                        nop.engine = inst.engine
                        nop.sync_info = bass_rust.SyncInfo(
                            on_wait=[w], on_update=[])
                        out.append(nop)
                    inst.sync_info = bass_rust.SyncInfo(
                        on_wait=[waits[-1]], on_update=list(si.on_update))
                out.append(inst)
            if changed:
                insts[:] = out
    return nc


def _make_consts(W_arr, b_arr):
    """Banded weights for the pair-interleaved layout, fp16.

    wb[tap][12s + 2*(3fw+ci) + q, 16s+co] = W[2*tap+q, fw, ci, co]
    (tap = which row pair, q = row parity inside the pair). Variants:
      std  [121|120, 144]  rows 0..119 (+bias row 120 on tap0)
      B0   [109|108, 144]  rows 12..119 shifted to 0 (left pad dropped)
      B14  [ 37| 36,  48]  rows 0..35, 3 outputs (right pad dropped)
    Bias b[co] is baked into row K of each tap-0 variant (multiplied by the
    strips' ones-row). Packed into one [121, 672] tensor.
    """
    wb = np.zeros((2, 120, 144), dtype=np.float32)
    for tap in range(2):
        for s_ in range(S):
            for fw in range(F):
                for ci in range(CIN):
                    for q in range(2):
                        wb[tap, 12 * s_ + 2 * (3 * fw + ci) + q,
                           16 * s_:16 * s_ + 16] = W_arr[2 * tap + q, fw, ci]
    bias = b_arr.reshape(-1).astype(np.float32)
    comb = np.zeros((121, 672), dtype=np.float16)
    # std
    comb[0:120, 0:144] = wb[0]
    comb[120, 0:144] = np.tile(bias, S)
    comb[0:120, 144:288] = wb[1]
    # B0: rows 12..119 -> 0..107
    comb[0:108, 288:432] = wb[0][12:120]
    comb[108, 288:432] = np.tile(bias, S)
    comb[0:108, 432:576] = wb[1][12:120]
    # B14: rows 0..35, 3 outputs
    comb[0:36, 576:624] = wb[0][0:36, 0:48]
    comb[36, 576:624] = np.tile(bias, 3)
    comb[0:36, 624:672] = wb[1][0:36, 0:48]
    return comb


def _make_strips(A_core):
    """Per-core input -> [121, 15*1040] fp16 strip tensor.

    G[img, re', c]: re' = re+1 (pairs -1..128), c = 2*flat + rowparity.
    Strip B = G[:, :, c0:c0+K] transposed to [K, re', img], ones at row K.
    """
    A16 = A_core.reshape(IMG, H, RW).astype(np.float16)
    G = np.zeros((IMG, NPAIR, IC), dtype=np.float16)
    G[:, 1:129, 0::2] = A16[:, 0::2, :]
    G[:, 1:129, 1::2] = A16[:, 1::2, :]
    out = np.zeros((121, NWB * NI), dtype=np.float16)
    for B in range(NWB):
        c0, K, _ = _bgeo(B)
        t = np.transpose(G[:, :, c0:c0 + K], (2, 1, 0)).reshape(K, NI)
        out[0:K, NI * B:NI * B + NI] = t
        out[K, NI * B:NI * B + NI] = 1.0
    return out


def _build_nc():
    nc = bass.Bass()
    a_in = nc.declare_dram_parameter("A", [121, NWB * NI], DT, isOutput=False)
    c_in = nc.declare_dram_parameter("consts", [121, 672], DT, isOutput=False)
    z_out = nc.declare_dram_parameter("Z", [IMG, HO, OUTROW], DT,
                                      isOutput=True)

    with tile.TileContext(nc) as tc, ExitStack() as ctx:
        consts = ctx.enter_context(tc.tile_pool(name="consts", bufs=1))
        spool = ctx.enter_context(tc.tile_pool(name="strips", bufs=1))
        opool = ctx.enter_context(tc.tile_pool(name="oacc", bufs=3))
        ppool = ctx.enter_context(
            tc.tile_pool(name="pconv", bufs=6, space="PSUM"))
        pw_pool = ctx.enter_context(
            tc.tile_pool(name="pwarm", bufs=1, space="PSUM"))

        # PE warmup: dummy matmuls during the initial DMA wait so the HAM
        # clock gate opens (1.2 -> 2.4 GHz) before the real work starts
        wtile = consts.tile([128, 640], DT, tag="wtile", name="wtile")
        nc.gpsimd.memset(wtile[:], 0.0)
        pwarm = pw_pool.tile([128, 512], DT32, tag="pwarm", name="pwarm")
        for _ in range(10):
            nc.tensor.matmul(pwarm[:], wtile[0:128, 0:128],
                             wtile[0:128, 128:640], start=True, stop=True)

        call = consts.tile([121, 672], DT, tag="call", name="call")
        nc.scalar.dma_start(out=call[:], in_=c_in[:])
        wstd = (call[0:121, 0:144], call[0:120, 144:288])
        wb0 = (call[0:109, 288:432], call[0:108, 432:576])
        wb14 = (call[0:37, 576:624], call[0:36, 624:672])

        strips = spool.tile([121, NWB * NI], DT, tag="strips", name="strips")
        a_r = a_in.rearrange("p (B i) -> p B i", B=NWB)
        s_r = strips[:].rearrange("p (B i) -> p B i", B=NWB)
        for q in range(4):
            nc.sync.dma_start(out=s_r[:, :, QS[q]:QS[q + 1]],
                              in_=a_r[:, :, QS[q]:QS[q + 1]])

        for b in range(NHB):
            h0 = 16 * b
            nh = 16 if b < 8 else 1
            m = 8 * nh
            oacc = opool.tile([128, OUTROW], DT, tag="oacc")
            for B in range(NWB):
                c0, K, N = _bgeo(B)
                w0, w1 = wb0 if B == 0 else (wb14 if B == NWB - 1 else wstd)
                i0 = NI * B + 8 * h0
                pc = ppool.tile([128, 144], DT32, tag="pc")
                nc.tensor.matmul(pc[0:m, 0:N], strips[0:K + 1, i0:i0 + m],
                                 w0[0:K + 1, 0:N], start=True, stop=False)
                nc.tensor.matmul(pc[0:m, 0:N],
                                 strips[0:K, i0 + 8:i0 + 8 + m],
                                 w1[0:K, 0:N], start=False, stop=True)
                # fused ReLU eviction, 2:1 DVE:ACT to balance engines
                dst = oacc[0:m, 144 * B:144 * B + N]
                if B % 3 == 2:
                    nc.scalar.activation(dst, pc[0:m, 0:N],
                                         mybir.ActivationFunctionType.Relu)
                else:
                    nc.vector.tensor_scalar_max(dst, pc[0:m, 0:N], 0.0)
            zdst = z_out[:, h0:h0 + nh, :].rearrange("i j c -> j i c")
            nc.scalar.dma_start(out=zdst, in_=oacc[0:m, :])

    _split_multi_waits(nc)
    return nc


_NC_CACHE = {}


def _get_nc():
    if "nc" not in _NC_CACHE:
        _NC_CACHE["nc"] = _build_nc()
    return _NC_CACHE["nc"]


def kernel(A_prev, W, b, _trace=False, _dt=None):
    A_prev = np.ascontiguousarray(A_prev, dtype=np.float32)
    W = np.asarray(W, dtype=np.float32)
    b = np.asarray(b, dtype=np.float32)
    comb = _make_consts(W, b)

    nc = _get_nc()
    in_maps = []
    for c in range(N_CORES):
        strips = _make_strips(A_prev[c * IMG:(c + 1) * IMG])
        in_maps.append({"A": strips, "consts": comb})

    res = run_bass_kernel_spmd(nc, in_maps, list(range(N_CORES)),
                               trace=_trace)
    out = np.concatenate(
        [res.results[c]["Z"].astype(np.float32).reshape(IMG, -1)
         for c in range(N_CORES)], axis=0)
    if _trace:
        return out, res
    return out


# revision 3
# speedup vs baseline: 1.4765x; 1.0905x over previous
"""TRN2 Bass kernel for OneLayerCNN: conv2d(4x4, stride 2, pad 2) + bias + ReLU.

Input  A_prev (64, 256, 256, 3) f32, W (4,4,3,16), b (1,1,1,16)
Output (64, 129*129*16) f32.

Data-parallel over 8 NeuronCores (8 images each). v3 design:

- Host pre-packs the input into matmul-ready fp16 "column strips": row PAIRS
  (2re, 2re+1) are column-interleaved (c = 2*(3x+ci) + rowparity), so one
  K<=121 band window spans TWO filter rows -> only 2 accumulating matmuls
  per output block (vs 4 with single-row banding), and the strips arrive
  transposed ([band-offset, (pair,img)]) so there are ZERO PE transposes
  and zero PSUM->SBUF transpose copies.
- The strips are shipped as 4 instance-GROUP arrays (group g = h-blocks
  2g, 2g+1, with the 16-instance seams duplicated) laid out so each group
  is ONE DMA with 8160B-contiguous per-partition descriptors, and each
  h-block's matmuls depend only on its own group's tile (fine-grained
  pipelining; PE starts after ~1MB, not after the full input).
- 15 w-blocks of 9 outputs (K=120+bias row, N=144), 9 h-blocks of 16 rows
  (M=128 = 16 h' x 8 img). Stationary operand = activation window (fp16 ->
  fast weight load), moving = banded weights.
- 3 w-blocks share one PSUM bank (3 x 144 cols = 1728B), evicted in one
  fused-ReLU op of [128, 432] (amortizes the per-op eviction overhead),
  alternating DVE/ACT.
- Bias rides a host-baked ones-row at partition K of each strip (tap-0
  matmul only). Zero pads are K-truncations of border blocks with
  host-shifted weight variants.
- fp16 everywhere off-chip (half the HBM traffic of f32); PSUM accumulates
  fp32; output is written fp16 and upcast to f32 on the host.
A post-pass splits multi-sem-wait instructions (walrus accepts one sync
wait per instruction). A short PE warmup during the initial DMA wait opens
the HAM clock gate before the real matmuls.
"""
import numpy as np
from contextlib import ExitStack

import concourse.bass as bass
import concourse.tile as tile
from concourse import mybir
from concourse.bass_utils import run_bass_kernel_spmd
import bass_rust

# ---------------- problem constants (hardcoded) ----------------
N_CORES = 8
IMG = 8              # images per core
H = 256
WID = 256
CIN = 3
F = 4
COUT = 16
HO = 129
WO = 129
RW = WID * CIN       # 768 floats per row
IC = 2 * RW          # 1536: row-pair interleaved width
S = 9                # w' outputs per w-block
NWB = 15             # w-blocks (14 full + 1 of 3 outputs)
NPAIR = 130          # row pairs incl. re=-1 and re=128 zero pads
NI = NPAIR * IMG     # 1040 instance columns total
NHB = 9              # h-blocks: 8 x 16 h' + 1 x 1 h'
OUTROW = WO * COUT   # 2064
NG = 4               # instance groups (h-block pairs + ragged)
GW = 272             # instance columns per group (264 used + seam slack)
GSTEP = 256          # group stride in global instance coords

DT = mybir.dt.float16
DT32 = mybir.dt.float32


def _bgeo(B):
    """w-block geometry: (c0 = window start in interleaved coords,
    K = band rows, N = matmul cols)."""
    c0 = max(0, 108 * B - 12)
    c1 = min(IC, 108 * B + 108)
    ns = min(S, WO - S * B)
    return c0, c1 - c0, ns * COUT


def _split_multi_waits(nc):
    """walrus accepts at most ONE sync wait per instruction; hoist extras
    onto NoOps inserted just before, same engine queue."""
    ctr = 0
    for f in nc.m.functions:
        for bb in f.blocks:
            insts = bb.instructions  # live list
            out = []
            changed = False
            for inst in insts:
                si = inst.sync_info
                if si is None:
                    out.append(inst)
                    continue
                waits = list(si.on_wait)
                if len(waits) > 1:
                    changed = True
                    for w in waits[:-1]:
                        ctr += 1
                        nop = mybir.InstNoOp(name=f"I-wsplit-{ctr}")
                        nop.engine = inst.engine
                        nop.sync_info = bass_rust.SyncInfo(
                            on_wait=[w], on_update=[])
                        out.append(nop)
                    inst.sync_info = bass_rust.SyncInfo(
                        on_wait=[waits[-1]], on_update=list(si.on_update))
                out.append(inst)
            if changed:
                insts[:] = out
    return nc


def _make_consts(W_arr, b_arr):
    """Banded weights for the pair-interleaved layout, fp16.

    wb[tap][12s + 2*(3fw+ci) + q, 16s+co] = W[2*tap+q, fw, ci, co]
    (tap = which row pair, q = row parity inside the pair). Variants:
      std  [121|120, 144]  rows 0..119 (+bias row 120 on tap0)
      B0   [109|108, 144]  rows 12..119 shifted to 0 (left pad dropped)
      B14  [ 37| 36,  48]  rows 0..35, 3 outputs (right pad dropped)
    Bias b[co] is baked into row K of each tap-0 variant (multiplied by the
    strips' ones-row). Packed into one [121, 672] tensor.
    """
    wb = np.zeros((2, 120, 144), dtype=np.float32)
    for tap in range(2):
        for s_ in range(S):
            for fw in range(F):
                for ci in range(CIN):
                    for q in range(2):
                        wb[tap, 12 * s_ + 2 * (3 * fw + ci) + q,
                           16 * s_:16 * s_ + 16] = W_arr[2 * tap + q, fw, ci]
    bias = b_arr.reshape(-1).astype(np.float32)
    comb = np.zeros((121, 672), dtype=np.float16)
    comb[0:120, 0:144] = wb[0]
    comb[120, 0:144] = np.tile(bias, S)
    comb[0:120, 144:288] = wb[1]
    comb[0:108, 288:432] = wb[0][12:120]
    comb[108, 288:432] = np.tile(bias, S)
    comb[0:108, 432:576] = wb[1][12:120]
    comb[0:36, 576:624] = wb[0][0:36, 0:48]
    comb[36, 576:624] = np.tile(bias, 3)
    comb[0:36, 624:672] = wb[1][0:36, 0:48]
    return comb


def _make_strips(A_core):
    """Per-core input -> [121, 4*15*272] fp16 group-major strip tensor.

    G[img, re', c]: re' = re+1 (pairs -1..128), c = 2*flat + rowparity.
    Strip B = G[:, :, c0:c0+K] transposed to [K, (re', img)], ones row at K.
    Group g holds instance columns [256g, 256g+272) of every strip,
    contiguous per partition: out[p, 4080*g + 272*B + i] .
    """
    A16 = A_core.reshape(IMG, H, RW).astype(np.float16)
    G = np.zeros((IMG, NPAIR, IC), dtype=np.float16)
    G[:, 1:129, 0::2] = A16[:, 0::2, :]
    G[:, 1:129, 1::2] = A16[:, 1::2, :]
    full = np.zeros((121, NWB, NI), dtype=np.float16)
    for B in range(NWB):
        c0, K, _ = _bgeo(B)
        full[0:K, B] = np.transpose(G[:, :, c0:c0 + K], (2, 1, 0)
                                    ).reshape(K, NI)
        full[K, B] = 1.0
    out = np.empty((121, NG, NWB, GW), dtype=np.float16)
    for g in range(NG):
        out[:, g] = full[:, :, GSTEP * g:GSTEP * g + GW]
    return np.ascontiguousarray(out.reshape(121, NG * NWB * GW))


def _build_nc():
    nc = bass.Bass()
    a_in = nc.declare_dram_parameter("A", [121, NG * NWB * GW], DT,
                                     isOutput=False)
    c_in = nc.declare_dram_parameter("consts", [121, 672], DT, isOutput=False)
    z_out = nc.declare_dram_parameter("Z", [IMG, HO, OUTROW], DT,
                                      isOutput=True)

    with tile.TileContext(nc) as tc, ExitStack() as ctx:
        consts = ctx.enter_context(tc.tile_pool(name="consts", bufs=1))
        spool = ctx.enter_context(tc.tile_pool(name="strips", bufs=NG))
        opool = ctx.enter_context(tc.tile_pool(name="oacc", bufs=3))
        ppool = ctx.enter_context(
            tc.tile_pool(name="pconv", bufs=4, space="PSUM"))
        pw_pool = ctx.enter_context(
            tc.tile_pool(name="pwarm", bufs=1, space="PSUM"))

        # PE warmup: dummy matmuls during the initial DMA wait so the HAM
        # clock gate opens (1.2 -> 2.4 GHz) before the real work starts
        wtile = consts.tile([128, 640], DT, tag="wtile", name="wtile")
        nc.gpsimd.memset(wtile[:], 0.0)
        pwarm = pw_pool.tile([128, 512], DT32, tag="pwarm", name="pwarm")
        for _ in range(8):
            nc.tensor.matmul(pwarm[:], wtile[0:128, 0:128],
                             wtile[0:128, 128:640], start=True, stop=True)

        call = consts.tile([121, 672], DT, tag="call", name="call")
        nc.scalar.dma_start(out=call[:], in_=c_in[:])
        wstd = (call[0:121, 0:144], call[0:120, 144:288])
        wb0 = (call[0:109, 288:432], call[0:108, 432:576])
        wb14 = (call[0:37, 576:624], call[0:36, 624:672])

        GSZ = NWB * GW   # 4080 cols per group
        sg = []
        for g in range(NG):
            t = spool.tile([121, GSZ], DT, tag=f"sg{g}", name=f"sg{g}")
            nc.sync.dma_start(out=t[:], in_=a_in[:, GSZ * g:GSZ * (g + 1)])
            sg.append(t)

        for b in range(NHB):
            g = min(b // 2, NG - 1)
            il = 128 * b - GSTEP * g      # 0 | 128 | 256 (b=8)
            nh = 16 if b < 8 else 1
            m = 8 * nh
            oacc = opool.tile([128, OUTROW], DT, tag="oacc")
            for E in range(5):           # 5 PSUM banks x 3 w-blocks
                pc = ppool.tile([128, 432], DT32, tag="pc")
                for j in range(3):
                    B = 3 * E + j
                    c0, K, N = _bgeo(B)
                    w0, w1 = wb0 if B == 0 else (
                        wb14 if B == NWB - 1 else wstd)
                    i0 = GW * B + il
                    nc.tensor.matmul(
                        pc[0:m, 144 * j:144 * j + N],
                        sg[g][0:K + 1, i0:i0 + m], w0[0:K + 1, 0:N],
                        start=True, stop=False)
                    nc.tensor.matmul(
                        pc[0:m, 144 * j:144 * j + N],
                        sg[g][0:K, i0 + 8:i0 + 8 + m], w1[0:K, 0:N],
                        start=False, stop=True)
                nE = 432 if E < 4 else 336
                dst = oacc[0:m, 432 * E:432 * E + nE]
                # fused ReLU eviction; alternate DVE/ACT to balance engines
                if E % 2 == 1:
                    nc.scalar.activation(dst, pc[0:m, 0:nE],
                                         mybir.ActivationFunctionType.Relu)
                else:
                    nc.vector.tensor_scalar_max(dst, pc[0:m, 0:nE], 0.0)
            h0 = 16 * b
            zdst = z_out[:, h0:h0 + nh, :].rearrange("i j c -> j i c")
            nc.scalar.dma_start(out=zdst, in_=oacc[0:m, :])

    _split_multi_waits(nc)
    return nc


_NC_CACHE = {}


def _get_nc():
    if "nc" not in _NC_CACHE:
        _NC_CACHE["nc"] = _build_nc()
    return _NC_CACHE["nc"]


def kernel(A_prev, W, b, _trace=False, _dt=None):
    A_prev = np.ascontiguousarray(A_prev, dtype=np.float32)
    W = np.asarray(W, dtype=np.float32)
    b = np.asarray(b, dtype=np.float32)
    comb = _make_consts(W, b)

    nc = _get_nc()
    in_maps = []
    for c in range(N_CORES):
        strips = _make_strips(A_prev[c * IMG:(c + 1) * IMG])
        in_maps.append({"A": strips, "consts": comb})

    res = run_bass_kernel_spmd(nc, in_maps, list(range(N_CORES)),
                               trace=_trace)
    out = np.concatenate(
        [res.results[c]["Z"].astype(np.float32).reshape(IMG, -1)
         for c in range(N_CORES)], axis=0)
    if _trace:
        return out, res
    return out


# revision 9
# speedup vs baseline: 2.0077x; 1.3598x over previous
"""TRN2 Bass kernel for OneLayerCNN: conv2d(4x4, stride 2, pad 2) + bias + ReLU.

Input  A_prev (64, 256, 256, 3) f32, W (4,4,3,16), b (1,1,1,16)
Output (64, 129*129*16) f32.

Data-parallel over 8 NeuronCores (8 images each). v3 design:

- Host pre-packs the input into matmul-ready fp16 "column strips": row PAIRS
  (2re, 2re+1) are column-interleaved (c = 2*(3x+ci) + rowparity), so one
  K<=121 band window spans TWO filter rows -> only 2 accumulating matmuls
  per output block (vs 4 with single-row banding), and the strips arrive
  transposed ([band-offset, (pair,img)]) so there are ZERO PE transposes
  and zero PSUM->SBUF transpose copies.
- The strips are shipped as 4 instance-GROUP arrays (group g = h-blocks
  2g, 2g+1, with the 16-instance seams duplicated) laid out so each group
  is ONE DMA with 8160B-contiguous per-partition descriptors, and each
  h-block's matmuls depend only on its own group's tile (fine-grained
  pipelining; PE starts after ~1MB, not after the full input).
- 15 w-blocks of 9 outputs (K=120+bias row, N=144), 9 h-blocks of 16 rows
  (M=128 = 16 h' x 8 img). Stationary operand = activation window (fp16 ->
  fast weight load), moving = banded weights.
- 3 w-blocks share one PSUM bank (3 x 144 cols = 1728B), evicted in one
  fused-ReLU op of [128, 432] (amortizes the per-op eviction overhead),
  alternating DVE/ACT.
- Bias rides a host-baked ones-row at partition K of each strip (tap-0
  matmul only). Zero pads are K-truncations of border blocks with
  host-shifted weight variants.
- fp16 everywhere off-chip (half the HBM traffic of f32); PSUM accumulates
  fp32; output is written fp16 and upcast to f32 on the host.
A post-pass splits multi-sem-wait instructions (walrus accepts one sync
wait per instruction). A short PE warmup during the initial DMA wait opens
the HAM clock gate before the real matmuls.
"""
import numpy as np
from contextlib import ExitStack

import concourse.bass as bass
import concourse.tile as tile
from concourse import mybir
from concourse.bass_utils import run_bass_kernel_spmd
import bass_rust

# ---------------- problem constants (hardcoded) ----------------
N_CORES = 8
IMG = 8              # images per core
H = 256
WID = 256
CIN = 3
F = 4
COUT = 16
HO = 129
WO = 129
RW = WID * CIN       # 768 floats per row
IC = 2 * RW          # 1536: row-pair interleaved width
S = 9                # w' outputs per w-block
NWB = 15             # w-blocks (14 full + 1 of 3 outputs)
NPAIR = 130          # row pairs incl. re=-1 and re=128 zero pads
NI = NPAIR * IMG     # 1040 instance columns total
NHB = 9              # h-blocks: 8 x 16 h' + 1 x 1 h'
OUTROW = WO * COUT   # 2064
NG = 4               # instance groups (h-block pairs + ragged)
GW = 272             # instance columns per group (264 used + seam slack)
GSTEP = 256          # group stride in global instance coords

DT = mybir.dt.float16
DT32 = mybir.dt.float32


def _bgeo(B):
    """w-block geometry: (c0 = window start in interleaved coords,
    K = band rows, N = matmul cols)."""
    c0 = max(0, 108 * B - 12)
    c1 = min(IC, 108 * B + 108)
    ns = min(S, WO - S * B)
    return c0, c1 - c0, ns * COUT


def _split_multi_waits(nc):
    """walrus accepts at most ONE sync wait per instruction; hoist extras
    onto NoOps inserted just before, same engine queue."""
    ctr = 0
    for f in nc.m.functions:
        for bb in f.blocks:
            insts = bb.instructions  # live list
            out = []
            changed = False
            for inst in insts:
                si = inst.sync_info
                if si is None:
                    out.append(inst)
                    continue
                waits = list(si.on_wait)
                if len(waits) > 1:
                    changed = True
                    for w in waits[:-1]:
                        ctr += 1
                        nop = mybir.InstNoOp(name=f"I-wsplit-{ctr}")
                        nop.engine = inst.engine
                        nop.sync_info = bass_rust.SyncInfo(
                            on_wait=[w], on_update=[])
                        out.append(nop)
                    inst.sync_info = bass_rust.SyncInfo(
                        on_wait=[waits[-1]], on_update=list(si.on_update))
                out.append(inst)
            if changed:
                insts[:] = out
    return nc


def _make_consts(W_arr, b_arr):
    """Banded weights for the pair-interleaved layout, fp16.

    wb[tap][12s + 2*(3fw+ci) + q, 16s+co] = W[2*tap+q, fw, ci, co]
    (tap = which row pair, q = row parity inside the pair). Variants:
      std  [121|120, 144]  rows 0..119 (+bias row 120 on tap0)
      B0   [109|108, 144]  rows 12..119 shifted to 0 (left pad dropped)
      B14  [ 37| 36,  48]  rows 0..35, 3 outputs (right pad dropped)
    Bias b[co] is baked into row K of each tap-0 variant (multiplied by the
    strips' ones-row). Packed into one [121, 672] tensor.
    """
    wb = np.zeros((2, 120, 144), dtype=np.float32)
    for tap in range(2):
        for s_ in range(S):
            for fw in range(F):
                for ci in range(CIN):
                    for q in range(2):
                        wb[tap, 12 * s_ + 2 * (3 * fw + ci) + q,
                           16 * s_:16 * s_ + 16] = W_arr[2 * tap + q, fw, ci]
    bias = b_arr.reshape(-1).astype(np.float32)
    # K is padded to 128 on device (FWL wants full-128 stationary operands);
    # rows >= the true K are ZERO here, which nullifies whatever sits in the
    # strip tiles' pad partitions.
    comb = np.zeros((128, 672), dtype=np.float16)
    comb[0:120, 0:144] = wb[0]
    comb[120, 0:144] = np.tile(bias, S)
    comb[0:120, 144:288] = wb[1]
    comb[0:108, 288:432] = wb[0][12:120]
    comb[108, 288:432] = np.tile(bias, S)
    comb[0:108, 432:576] = wb[1][12:120]
    comb[0:36, 576:624] = wb[0][0:36, 0:48]
    comb[36, 576:624] = np.tile(bias, 3)
    comb[0:36, 624:672] = wb[1][0:36, 0:48]
    return comb


def _make_strips(A_core):
    """Per-core input -> [121, 4*15*272] fp16 group-major strip tensor.

    G[img, re', c]: re' = re+1 (pairs -1..128), c = 2*flat + rowparity.
    Strip B = G[:, :, c0:c0+K] transposed to [K, (re', img)], ones row at K.
    Group g holds instance columns [256g, 256g+272) of every strip,
    contiguous per partition: out[p, 4080*g + 272*B + i] .
    """
    A16 = A_core.reshape(IMG, H, RW).astype(np.float16)
    G = np.zeros((IMG, NPAIR, IC), dtype=np.float16)
    G[:, 1:129, 0::2] = A16[:, 0::2, :]
    G[:, 1:129, 1::2] = A16[:, 1::2, :]
    full = np.zeros((128, NWB, NI), dtype=np.float16)
    for B in range(NWB):
        c0, K, _ = _bgeo(B)
        full[0:K, B] = np.transpose(G[:, :, c0:c0 + K], (2, 1, 0)
                                    ).reshape(K, NI)
        full[K, B] = 1.0
    out = np.empty((128, NG, NWB, GW), dtype=np.float16)
    for g in range(NG):
        out[:, g] = full[:, :, GSTEP * g:GSTEP * g + GW]
    return np.ascontiguousarray(out.reshape(128, NG * NWB * GW))


def _build_nc():
    nc = bass.Bass()
    a_in = nc.declare_dram_parameter("A", [128, NG * NWB * GW], DT,
                                     isOutput=False)
    c_in = nc.declare_dram_parameter("consts", [128, 672], DT, isOutput=False)
    z_out = nc.declare_dram_parameter("Z", [IMG, HO, OUTROW], DT,
                                      isOutput=True)

    with tile.TileContext(nc) as tc, ExitStack() as ctx:
        consts = ctx.enter_context(tc.tile_pool(name="consts", bufs=1))
        spool = ctx.enter_context(tc.tile_pool(name="strips", bufs=NG))
        opool = ctx.enter_context(tc.tile_pool(name="oacc", bufs=3))
        ppool = ctx.enter_context(
            tc.tile_pool(name="pconv", bufs=6, space="PSUM"))
        pw_pool = ctx.enter_context(
            tc.tile_pool(name="pwarm", bufs=1, space="PSUM"))

        GSZ = NWB * GW   # 4080 cols per group
        # input strips first: 3 sub-DMAs per group (5 strips each) so the
        # first matmuls are gated on ~0.33MB, not the whole input
        sg = []
        for g in range(NG):
            t = spool.tile([128, GSZ], DT, tag=f"sg{g}", name=f"sg{g}")
            for s3 in range(3):
                cl = GW * 5 * s3
                nc.sync.dma_start(
                    out=t[:, cl:cl + GW * 5],
                    in_=a_in[:, GSZ * g + cl:GSZ * g + cl + GW * 5])
            sg.append(t)

        call = consts.tile([128, 672], DT, tag="call", name="call")
        nc.scalar.dma_start(out=call[:], in_=c_in[:])
        # K padded to 128: weight pad rows are zero, so strip pad partitions
        # (121..127, memset below) contribute nothing
        wstd = (call[:, 0:144], call[:, 144:288])
        wb0 = (call[:, 288:432], call[:, 432:576])
        wb14 = (call[:, 576:624], call[:, 624:672])

        # PE warmup: dummy matmuls during the initial DMA wait so the HAM
        # clock gate opens (1.2 -> 2.4 GHz) before the real work starts
        wtile = consts.tile([128, 640], DT, tag="wtile", name="wtile")
        nc.gpsimd.memset(wtile[:], 0.0)
        pwarm = pw_pool.tile([128, 512], DT32, tag="pwarm", name="pwarm")
        for _ in range(8):
            nc.tensor.matmul(pwarm[:], wtile[0:128, 0:128],
                             wtile[0:128, 128:640], start=True, stop=True)

        ev = 0
        for b in range(NHB):
            g = min(b // 2, NG - 1)
            il = 128 * b - GSTEP * g      # 0 | 128 | 256 (b=8)
            nh = 16 if b < 8 else 1
            m = 8 * nh
            oacc = opool.tile([128, OUTROW], DT, tag="oacc")
            for E in range(5):           # 5 PSUM banks x 3 w-blocks
                pc = ppool.tile([128, 432], DT32, tag="pc")
                for j in range(3):
                    B = 3 * E + j
                    _, _, N = _bgeo(B)
                    w0, w1 = wb0 if B == 0 else (
                        wb14 if B == NWB - 1 else wstd)
                    i0 = GW * B + il
                    nc.tensor.matmul(
                        pc[0:m, 144 * j:144 * j + N],
                        sg[g][0:128, i0:i0 + m], w0[0:128, 0:N],
                        start=True, stop=False)
                    nc.tensor.matmul(
                        pc[0:m, 144 * j:144 * j + N],
                        sg[g][0:128, i0 + 8:i0 + 8 + m], w1[0:128, 0:N],
                        start=False, stop=True)
                nE = 432 if E < 4 else 336
                dst = oacc[0:m, 432 * E:432 * E + nE]
                # fused ReLU eviction; alternate DVE/ACT by global parity
                if ev % 2 == 1:
                    nc.scalar.activation(dst, pc[0:m, 0:nE],
                                         mybir.ActivationFunctionType.Relu)
                else:
                    nc.vector.tensor_scalar_max(dst, pc[0:m, 0:nE], 0.0)
                ev += 1
            h0 = 16 * b
            zdst = z_out[:, h0:h0 + nh, :].rearrange("i j c -> j i c")
            nc.scalar.dma_start(out=zdst, in_=oacc[0:m, :])

    _split_multi_waits(nc)
    return nc


_NC_CACHE = {}


def _get_nc():
    if "nc" not in _NC_CACHE:
        _NC_CACHE["nc"] = _build_nc()
    return _NC_CACHE["nc"]


def kernel(A_prev, W, b, _trace=False, _dt=None):
    A_prev = np.ascontiguousarray(A_prev, dtype=np.float32)
    W = np.asarray(W, dtype=np.float32)
    b = np.asarray(b, dtype=np.float32)
    comb = _make_consts(W, b)

    nc = _get_nc()
    in_maps = []
    for c in range(N_CORES):
        strips = _make_strips(A_prev[c * IMG:(c + 1) * IMG])
        in_maps.append({"A": strips, "consts": comb})

    res = run_bass_kernel_spmd(nc, in_maps, list(range(N_CORES)),
                               trace=_trace)
    out = np.concatenate(
        [res.results[c]["Z"].astype(np.float32).reshape(IMG, -1)
         for c in range(N_CORES)], axis=0)
    if _trace:
        return out, res
    return out


# revision 14
# speedup vs baseline: 2.1685x; 1.0800x over previous
"""TRN2 Bass kernel for OneLayerCNN: conv2d(4x4, stride 2, pad 2) + bias + ReLU.

Input  A_prev (64, 256, 256, 3) f32, W (4,4,3,16), b (1,1,1,16)
Output (64, 129*129*16) f32.

Data-parallel over 8 NeuronCores (8 images each). v3 design:

- Host pre-packs the input into matmul-ready fp16 "column strips": row PAIRS
  (2re, 2re+1) are column-interleaved (c = 2*(3x+ci) + rowparity), so one
  K<=121 band window spans TWO filter rows -> only 2 accumulating matmuls
  per output block (vs 4 with single-row banding), and the strips arrive
  transposed ([band-offset, (pair,img)]) so there are ZERO PE transposes
  and zero PSUM->SBUF transpose copies.
- The strips are shipped as 4 instance-GROUP arrays (group g = h-blocks
  2g, 2g+1, with the 16-instance seams duplicated) laid out so each group
  is ONE DMA with 8160B-contiguous per-partition descriptors, and each
  h-block's matmuls depend only on its own group's tile (fine-grained
  pipelining; PE starts after ~1MB, not after the full input).
- 15 w-blocks of 9 outputs (K=120+bias row, N=144), 9 h-blocks of 16 rows
  (M=128 = 16 h' x 8 img). Stationary operand = activation window (fp16 ->
  fast weight load), moving = banded weights.
- 3 w-blocks share one PSUM bank (3 x 144 cols = 1728B), evicted in one
  fused-ReLU op of [128, 432] (amortizes the per-op eviction overhead),
  alternating DVE/ACT.
- Bias rides a host-baked ones-row at partition K of each strip (tap-0
  matmul only). Zero pads are K-truncations of border blocks with
  host-shifted weight variants.
- fp16 everywhere off-chip (half the HBM traffic of f32); PSUM accumulates
  fp32; output is written fp16 and upcast to f32 on the host.
A post-pass splits multi-sem-wait instructions (walrus accepts one sync
wait per instruction). A short PE warmup during the initial DMA wait opens
the HAM clock gate before the real matmuls.
"""
import numpy as np
from contextlib import ExitStack

import concourse.bass as bass
import concourse.tile as tile
from concourse import mybir
from concourse.bass_utils import run_bass_kernel_spmd
import bass_rust

# ---------------- problem constants (hardcoded) ----------------
N_CORES = 8
IMG = 8              # images per core
H = 256
WID = 256
CIN = 3
F = 4
COUT = 16
HO = 129
WO = 129
RW = WID * CIN       # 768 floats per row
IC = 2 * RW          # 1536: row-pair interleaved width
S = 9                # w' outputs per w-block
NWB = 15             # w-blocks (14 full + 1 of 3 outputs)
NPAIR = 130          # row pairs incl. re=-1 and re=128 zero pads
NI = NPAIR * IMG     # 1040 instance columns total
NHB = 9              # h-blocks: 8 x 16 h' + 1 x 1 h'
OUTROW = WO * COUT   # 2064
NG = 4               # instance groups (h-block pairs; group 0 also holds b=8)
GW = 272             # instance columns per group (264 used + seam slack)
GW0 = 288            # group 0: + the 16 ragged b=8 instances (1024..1039)
GSTEP = 256          # group stride in global instance coords

DT = mybir.dt.float16
DT32 = mybir.dt.float32


def _bgeo(B):
    """w-block geometry: (c0 = window start in interleaved coords,
    K = band rows, N = matmul cols)."""
    c0 = max(0, 108 * B - 12)
    c1 = min(IC, 108 * B + 108)
    ns = min(S, WO - S * B)
    return c0, c1 - c0, ns * COUT


def _split_multi_waits(nc):
    """walrus accepts at most ONE sync wait per instruction; hoist extras
    onto NoOps inserted just before, same engine queue."""
    ctr = 0
    for f in nc.m.functions:
        for bb in f.blocks:
            insts = bb.instructions  # live list
            out = []
            changed = False
            for inst in insts:
                si = inst.sync_info
                if si is None:
                    out.append(inst)
                    continue
                waits = list(si.on_wait)
                if len(waits) > 1:
                    changed = True
                    for w in waits[:-1]:
                        ctr += 1
                        nop = mybir.InstNoOp(name=f"I-wsplit-{ctr}")
                        nop.engine = inst.engine
                        nop.sync_info = bass_rust.SyncInfo(
                            on_wait=[w], on_update=[])
                        out.append(nop)
                    inst.sync_info = bass_rust.SyncInfo(
                        on_wait=[waits[-1]], on_update=list(si.on_update))
                out.append(inst)
            if changed:
                insts[:] = out
    return nc


def _make_consts(W_arr, b_arr):
    """Banded weights for the pair-interleaved layout, fp16.

    wb[tap][12s + 2*(3fw+ci) + q, 16s+co] = W[2*tap+q, fw, ci, co]
    (tap = which row pair, q = row parity inside the pair). Variants:
      std  [121|120, 144]  rows 0..119 (+bias row 120 on tap0)
      B0   [109|108, 144]  rows 12..119 shifted to 0 (left pad dropped)
      B14  [ 37| 36,  48]  rows 0..35, 3 outputs (right pad dropped)
    Bias b[co] is baked into row K of each tap-0 variant (multiplied by the
    strips' ones-row). Packed into one [121, 672] tensor.
    """
    wb = np.zeros((2, 120, 144), dtype=np.float32)
    for tap in range(2):
        for s_ in range(S):
            for fw in range(F):
                for ci in range(CIN):
                    for q in range(2):
                        wb[tap, 12 * s_ + 2 * (3 * fw + ci) + q,
                           16 * s_:16 * s_ + 16] = W_arr[2 * tap + q, fw, ci]
    bias = b_arr.reshape(-1).astype(np.float32)
    # K is padded to 128 on device (FWL wants full-128 stationary operands);
    # rows >= the true K are ZERO here, which nullifies whatever sits in the
    # strip tiles' pad partitions.
    comb = np.zeros((128, 672), dtype=np.float16)
    comb[0:120, 0:144] = wb[0]
    comb[120, 0:144] = np.tile(bias, S)
    comb[0:120, 144:288] = wb[1]
    comb[0:108, 288:432] = wb[0][12:120]
    comb[108, 288:432] = np.tile(bias, S)
    comb[0:108, 432:576] = wb[1][12:120]
    comb[0:36, 576:624] = wb[0][0:36, 0:48]
    comb[36, 576:624] = np.tile(bias, 3)
    comb[0:36, 624:672] = wb[1][0:36, 0:48]
    return comb


def _make_strips(A_core):
    """Per-core input -> [121, 4*15*272] fp16 group-major strip tensor.

    G[img, re', c]: re' = re+1 (pairs -1..128), c = 2*flat + rowparity.
    Strip B = G[:, :, c0:c0+K] transposed to [K, (re', img)], ones row at K.
    Group g holds instance columns [256g, 256g+272) of every strip,
    contiguous per partition: out[p, 4080*g + 272*B + i] .
    """
    A16 = A_core.reshape(IMG, H, RW).astype(np.float16)
    G = np.zeros((IMG, NPAIR, IC), dtype=np.float16)
    G[:, 1:129, 0::2] = A16[:, 0::2, :]
    G[:, 1:129, 1::2] = A16[:, 1::2, :]
    full = np.zeros((128, NWB, NI), dtype=np.float16)
    for B in range(NWB):
        c0, K, _ = _bgeo(B)
        full[0:K, B] = np.transpose(G[:, :, c0:c0 + K], (2, 1, 0)
                                    ).reshape(K, NI)
        full[K, B] = 1.0
    parts = []
    g0 = np.concatenate([full[:, :, 0:GW],
                         full[:, :, NI - 16:NI]], axis=2)   # + b=8 insts
    parts.append(g0.reshape(128, NWB * GW0))
    for g in range(1, NG):
        parts.append(np.ascontiguousarray(
            full[:, :, GSTEP * g:GSTEP * g + GW]).reshape(128, NWB * GW))
    return np.ascontiguousarray(np.concatenate(parts, axis=1))


def _build_nc():
    nc = bass.Bass()
    NCOL = NWB * (GW0 + (NG - 1) * GW)
    a_in = nc.declare_dram_parameter("A", [128, NCOL], DT, isOutput=False)
    c_in = nc.declare_dram_parameter("consts", [128, 672], DT, isOutput=False)
    z_out = nc.declare_dram_parameter("Z", [IMG, HO, OUTROW], DT,
                                      isOutput=True)

    with tile.TileContext(nc) as tc, ExitStack() as ctx:
        consts = ctx.enter_context(tc.tile_pool(name="consts", bufs=1))
        spool = ctx.enter_context(tc.tile_pool(name="strips", bufs=1))
        opool = ctx.enter_context(tc.tile_pool(name="oacc", bufs=3))
        ppool = ctx.enter_context(
            tc.tile_pool(name="pconv", bufs=7, space="PSUM"))
        pw_pool = ctx.enter_context(
            tc.tile_pool(name="pwarm", bufs=1, space="PSUM"))

        # input strips first: 3 sub-tiles per group (5 strips each), each its
        # own tile so matmuls gate on exactly the 0.35MB they read
        sgt = []       # sgt[g][s3] tile, strip B at cols (B%5)*GWg
        off = 0
        for g in range(NG):
            GWg = GW0 if g == 0 else GW
            row = []
            for s3 in range(3):
                t = spool.tile([128, 5 * GWg], DT, tag=f"sg{g}_{s3}",
                               name=f"sg{g}_{s3}")
                nc.sync.dma_start(out=t[:], in_=a_in[:, off:off + 5 * GWg])
                off += 5 * GWg
                row.append(t)
            sgt.append(row)

        call = consts.tile([128, 672], DT, tag="call", name="call")
        nc.scalar.dma_start(out=call[:], in_=c_in[:])
        # K padded to 128: weight pad rows are zero, so whatever sits in the
        # strip pad partitions contributes nothing
        wstd = (call[:, 0:144], call[:, 144:288])
        wb0 = (call[:, 288:432], call[:, 432:576])
        wb14 = (call[:, 576:624], call[:, 624:672])

        # PE warmup: dummy matmuls during the initial DMA wait so the HAM
        # clock gate opens (1.2 -> 2.4 GHz) before the real work starts
        wtile = consts.tile([128, 640], DT, tag="wtile", name="wtile")
        nc.gpsimd.memset(wtile[:], 0.0)
        pwarm = pw_pool.tile([128, 512], DT32, tag="pwarm", name="pwarm")
        for _ in range(6):
            nc.tensor.matmul(pwarm[:], wtile[0:128, 0:128],
                             wtile[0:128, 128:640], start=True, stop=True)

        ev = 0
        # ragged h-block (b=8, h'=128) first: keeps it off the tail; its
        # instances are duplicated at cols [272:288) of group 0
        for b in [8] + list(range(8)):
            g = 0 if b == 8 else b // 2
            GWg = GW0 if g == 0 else GW
            il = 272 if b == 8 else 128 * b - GSTEP * g   # 0 | 128
            nh = 16 if b < 8 else 1
            m = 8 * nh
            oacc = opool.tile([128, OUTROW], DT, tag="oacc")
            for E in range(5):           # 5 PSUM banks x 3 w-blocks
                pc = ppool.tile([128, 432], DT32, tag="pc")
                for j in range(3):
                    B = 3 * E + j
                    _, _, N = _bgeo(B)
                    w0, w1 = wb0 if B == 0 else (
                        wb14 if B == NWB - 1 else wstd)
                    st = sgt[g][B // 5]
                    i0 = GWg * (B % 5) + il
                    nc.tensor.matmul(
                        pc[0:m, 144 * j:144 * j + N],
                        st[0:128, i0:i0 + m], w0[0:128, 0:N],
                        start=True, stop=False)
                    nc.tensor.matmul(
                        pc[0:m, 144 * j:144 * j + N],
                        st[0:128, i0 + 8:i0 + 8 + m], w1[0:128, 0:N],
                        start=False, stop=True)
                nE = 432 if E < 4 else 336
                dst = oacc[0:m, 432 * E:432 * E + nE]
                # fused ReLU eviction; alternate DVE/ACT by global parity
                if ev % 2 == 1:
                    nc.scalar.activation(dst, pc[0:m, 0:nE],
                                         mybir.ActivationFunctionType.Relu)
                else:
                    nc.vector.tensor_scalar_max(dst, pc[0:m, 0:nE], 0.0)
                ev += 1
            h0 = 16 * b
            zdst = z_out[:, h0:h0 + nh, :].rearrange("i j c -> j i c")
            # output DMAs ride the sync queue: the scalar queue's sequencer
            # runs the ACT evictions and must not stall on DGE work
            nc.sync.dma_start(out=zdst, in_=oacc[0:m, :])

    _split_multi_waits(nc)
    return nc


_NC_CACHE = {}


def _get_nc():
    if "nc" not in _NC_CACHE:
        _NC_CACHE["nc"] = _build_nc()
    return _NC_CACHE["nc"]


def kernel(A_prev, W, b, _trace=False, _dt=None):
    A_prev = np.ascontiguousarray(A_prev, dtype=np.float32)
    W = np.asarray(W, dtype=np.float32)
    b = np.asarray(b, dtype=np.float32)
    comb = _make_consts(W, b)

    nc = _get_nc()
    in_maps = []
    for c in range(N_CORES):
        strips = _make_strips(A_prev[c * IMG:(c + 1) * IMG])
        in_maps.append({"A": strips, "consts": comb})

    res = run_bass_kernel_spmd(nc, in_maps, list(range(N_CORES)),
                               trace=_trace)
    out = np.concatenate(
        [res.results[c]["Z"].astype(np.float32).reshape(IMG, -1)
         for c in range(N_CORES)], axis=0)
    if _trace:
        return out, res
    return out
